# revision 7
# baseline (speedup 1.0000x reference)
import os
import numpy as np
import jax
import jax.numpy as jnp
from jax.sharding import Mesh, PartitionSpec as P
from jax.experimental.shard_map import shard_map
from functools import partial

# ---- constants (hardcoded from the problem spec) ----
N = 1_024_000
G = 4000
NPD = 256
LV = 8
E = 1_000_000
KEX = 50
D = 8
SLOPE = 0.01
NC = 8
NLOC = N // NC      # 128000 nodes per core
GLOC = G // NC      # 500 dags per core


def _mlp(params, x):
    n = len(params)
    for i, (W, b) in enumerate(params):
        x = x @ W + b
        if i < n - 1:
            x = jax.nn.leaky_relu(x, SLOPE)
    return x


def _bucket_edges(level_src, level_dst, level_mask):
    """Host-side: bucket each level's edges by the owner core of src (the
    aggregation target). Drop edges whose target has mask==0 at that level
    (their aggregate is multiplied by 0 in the residual update, so they are
    dead work). Pad all (level, core) lists to one common length."""
    src = np.asarray(level_src)
    dst = np.asarray(level_dst)
    msk = np.asarray(level_mask) > 0.5
    owner = src // NLOC
    keep = msk[np.arange(LV)[:, None], src]          # [LV, E] target-mask alive
    counts = np.zeros((LV, NC), np.int64)
    buckets = {}
    for l in range(LV):
        for c in range(NC):
            sel = np.nonzero((owner[l] == c) & keep[l])[0]
            buckets[(l, c)] = sel
            counts[l, c] = len(sel)
    emax = int(counts.max())
    emax = ((emax + 127) // 128) * 128
    s_out = np.zeros((LV, NC, emax), np.int32)       # local target id (padded -> NLOC sink)
    d_out = np.zeros((LV, NC, emax), np.int32)       # global source node
    for l in range(LV):
        for c in range(NC):
            sel = buckets[(l, c)]
            n = len(sel)
            s_out[l, c, :n] = src[l, sel] - c * NLOC
            s_out[l, c, n:] = NLOC                   # sink row (dropped)
            d_out[l, c, :n] = dst[l, sel]
    return s_out, d_out


def kernel(x, ptr, level_src, level_dst, level_mask,
           p_node_prep, p_node_msg, p_node_update, p_dag_msg,
           p_glob_msg, p_node_score, p_dag_score, num_executors):
    x = np.asarray(x, np.float32)
    level_mask = np.asarray(level_mask, np.float32)
    k = int(num_executors)

    es, ed = _bucket_edges(level_src, level_dst, level_mask)

    params = [tuple((jnp.asarray(W), jnp.asarray(b)) for (W, b) in p)
              for p in (p_node_prep, p_node_msg, p_node_update, p_dag_msg,
                        p_glob_msg, p_node_score, p_dag_score)]
    (pp, pm, pu, pd, pg, pns, pds) = params

    devs = np.array(jax.devices()[:NC])
    mesh = Mesh(devs, ("i",))

    xs = x.reshape(NC, NLOC, 5)
    ms = level_mask.reshape(LV, NC, NLOC).transpose(1, 0, 2)     # [NC, LV, NLOC]
    es_s = es.transpose(1, 0, 2)                                  # [NC, LV, Emax]
    ed_s = ed.transpose(1, 0, 2)

    exec_feat = np.tile(np.arange(k, dtype=np.float32), GLOC)[:, None]  # [GLOC*K, 1]

    @partial(shard_map, mesh=mesh,
             in_specs=(P("i"), P("i"), P("i"), P("i")),
             out_specs=(P("i"), P("i")))
    def run(xl, ml, esl, edl):
        xl = xl[0]; ml = ml[0]; esl = esl[0]; edl = edl[0]
        h = _mlp(pp, xl)                                          # [NLOC, 8]
        for l in range(LV):
            y = _mlp(pm, h)                                       # [NLOC, 8]
            y_full = jax.lax.all_gather(y, "i", tiled=True)       # [N, 8]
            emax = edl.shape[1]
            nch = 8
            csz = emax // nch
            agg = jnp.zeros((NLOC + 1, D), jnp.float32)
            for ci in range(nch):
                sl = slice(ci * csz, (ci + 1) * csz if ci < nch - 1 else emax)
                msgs = jnp.take(y_full, edl[l, sl], axis=0)
                agg = agg.at[esl[l, sl]].add(msgs)
            agg = agg[:NLOC]
            upd = _mlp(pu, agg)
            h = h + ml[l][:, None] * upd
        dmsg = _mlp(pd, jnp.concatenate([xl, h], axis=1))         # [NLOC, 8]
        dsum = dmsg.reshape(GLOC, NPD, D).sum(axis=1)             # [GLOC, 8]
        gl = _mlp(pg, dsum).sum(axis=0, keepdims=True)            # [1, 8]
        glob = jax.lax.psum(gl, "i")                              # [1, 8]
        dag_b = jnp.broadcast_to(dsum[:, None, :], (GLOC, NPD, D)).reshape(NLOC, D)
        nin = jnp.concatenate([xl, h, dag_b,
                               jnp.broadcast_to(glob, (NLOC, D))], axis=1)
        nsc = _mlp(pns, nin)[:, 0]                                # [NLOC]
        dag_feat = xl.reshape(GLOC, NPD, 5)[:, 0, 0:3]            # [GLOC, 3]
        merged = jnp.concatenate([dag_feat, dsum], axis=1)        # [GLOC, 11]
        mrep = jnp.broadcast_to(merged[:, None, :], (GLOC, k, 11)).reshape(GLOC * k, 11)
        din = jnp.concatenate([mrep, jnp.broadcast_to(glob, (GLOC * k, D)),
                               jnp.asarray(exec_feat)], axis=1)
        dsc = _mlp(pds, din)[:, 0].reshape(GLOC, k)               # [GLOC, K]
        return nsc[None], dsc[None]

    sh = jax.sharding.NamedSharding(mesh, P("i"))

    def put(a):
        a = np.ascontiguousarray(a)
        shards = [jax.device_put(a[i:i + 1], devs[i]) for i in range(NC)]
        return jax.make_array_from_single_device_arrays(a.shape, sh, shards)

    nsc, dsc = run(put(xs), put(ms), put(es_s), put(ed_s))
    node_scores = np.asarray(nsc).reshape(N)
    dag_scores = np.asarray(dsc).reshape(G, k)
    return node_scores, dag_scores


# revision 8
# speedup vs baseline: 1.0702x; 1.0702x over previous
import os
import numpy as np
import jax
import jax.numpy as jnp
from jax.sharding import Mesh, PartitionSpec as P
from jax.experimental.shard_map import shard_map
from functools import partial

# ---- constants (hardcoded from the problem spec) ----
N = 1_024_000
G = 4000
NPD = 256
LV = 8
E = 1_000_000
KEX = 50
D = 8
SLOPE = 0.01
NC = 8
NLOC = N // NC      # 128000 nodes per core
GLOC = G // NC      # 500 dags per core


def _mlp(params, x):
    n = len(params)
    for i, (W, b) in enumerate(params):
        x = x @ W + b
        if i < n - 1:
            x = jax.nn.leaky_relu(x, SLOPE)
    return x


def _bucket_edges(level_src, level_dst, level_mask):
    """Host-side: bucket each level's edges by the owner core of src (the
    aggregation target). Drop edges whose target has mask==0 at that level
    (their aggregate is multiplied by 0 in the residual update, so they are
    dead work). Pad all (level, core) lists to one common length."""
    src = np.asarray(level_src)
    dst = np.asarray(level_dst)
    msk = np.asarray(level_mask) > 0.5
    owner = src // NLOC
    keep = msk[np.arange(LV)[:, None], src]          # [LV, E] target-mask alive
    counts = np.zeros((LV, NC), np.int64)
    buckets = {}
    for l in range(LV):
        for c in range(NC):
            sel = np.nonzero((owner[l] == c) & keep[l])[0]
            buckets[(l, c)] = sel
            counts[l, c] = len(sel)
    emax = int(counts.max())
    emax = ((emax + 127) // 128) * 128
    s_out = np.zeros((LV, NC, emax), np.int32)       # local target id (padded -> NLOC sink)
    d_out = np.zeros((LV, NC, emax), np.int32)       # global source node
    for l in range(LV):
        for c in range(NC):
            sel = buckets[(l, c)]
            n = len(sel)
            s_out[l, c, :n] = src[l, sel] - c * NLOC
            s_out[l, c, n:] = NLOC                   # sink row (dropped)
            d_out[l, c, :n] = dst[l, sel]
    return s_out, d_out


def kernel(x, ptr, level_src, level_dst, level_mask,
           p_node_prep, p_node_msg, p_node_update, p_dag_msg,
           p_glob_msg, p_node_score, p_dag_score, num_executors):
    import time as _t
    _t0 = _t.time()
    x = np.asarray(x, np.float32)
    level_mask = np.asarray(level_mask, np.float32)
    k = int(num_executors)

    es, ed = _bucket_edges(level_src, level_dst, level_mask)
    print("[prep] bucket", _t.time() - _t0, flush=True)

    params = [tuple((jnp.asarray(W), jnp.asarray(b)) for (W, b) in p)
              for p in (p_node_prep, p_node_msg, p_node_update, p_dag_msg,
                        p_glob_msg, p_node_score, p_dag_score)]
    (pp, pm, pu, pd, pg, pns, pds) = params

    devs = np.array(jax.devices()[:NC])
    mesh = Mesh(devs, ("i",))

    xs = x.reshape(NC, NLOC, 5)
    ms = level_mask.reshape(LV, NC, NLOC).transpose(1, 0, 2)     # [NC, LV, NLOC]
    es_s = es.transpose(1, 0, 2)                                  # [NC, LV, Emax]
    ed_s = ed.transpose(1, 0, 2)

    exec_feat = np.tile(np.arange(k, dtype=np.float32), GLOC)[:, None]  # [GLOC*K, 1]

    @partial(shard_map, mesh=mesh,
             in_specs=(P("i"), P("i"), P("i"), P("i")),
             out_specs=(P("i"), P("i")))
    def run(xl, ml, esl, edl):
        xl = xl[0]; ml = ml[0]; esl = esl[0]; edl = edl[0]
        h = _mlp(pp, xl)                                          # [NLOC, 8]
        for l in range(LV):
            y = _mlp(pm, h)                                       # [NLOC, 8]
            y_full = jax.lax.all_gather(y, "i", tiled=True)       # [N, 8]
            emax = edl.shape[1]
            nch = 8
            csz = emax // nch
            agg = jnp.zeros((NLOC + 1, D), jnp.float32)
            for ci in range(nch):
                sl = slice(ci * csz, (ci + 1) * csz if ci < nch - 1 else emax)
                msgs = jnp.take(y_full, edl[l, sl], axis=0)
                agg = agg.at[esl[l, sl]].add(msgs)
            agg = agg[:NLOC]
            upd = _mlp(pu, agg)
            h = h + ml[l][:, None] * upd
        dmsg = _mlp(pd, jnp.concatenate([xl, h], axis=1))         # [NLOC, 8]
        dsum = dmsg.reshape(GLOC, NPD, D).sum(axis=1)             # [GLOC, 8]
        gl = _mlp(pg, dsum).sum(axis=0, keepdims=True)            # [1, 8]
        glob = jax.lax.psum(gl, "i")                              # [1, 8]
        dag_b = jnp.broadcast_to(dsum[:, None, :], (GLOC, NPD, D)).reshape(NLOC, D)
        nin = jnp.concatenate([xl, h, dag_b,
                               jnp.broadcast_to(glob, (NLOC, D))], axis=1)
        nsc = _mlp(pns, nin)[:, 0]                                # [NLOC]
        dag_feat = xl.reshape(GLOC, NPD, 5)[:, 0, 0:3]            # [GLOC, 3]
        merged = jnp.concatenate([dag_feat, dsum], axis=1)        # [GLOC, 11]
        mrep = jnp.broadcast_to(merged[:, None, :], (GLOC, k, 11)).reshape(GLOC * k, 11)
        din = jnp.concatenate([mrep, jnp.broadcast_to(glob, (GLOC * k, D)),
                               jnp.asarray(exec_feat)], axis=1)
        dsc = _mlp(pds, din)[:, 0].reshape(GLOC, k)               # [GLOC, K]
        return nsc[None], dsc[None]

    sh = jax.sharding.NamedSharding(mesh, P("i"))

    def put(a):
        a = np.ascontiguousarray(a)
        shards = [jax.device_put(a[i:i + 1], devs[i]) for i in range(NC)]
        return jax.make_array_from_single_device_arrays(a.shape, sh, shards)

    _t1 = _t.time()
    a1, a2, a3, a4 = put(xs), put(ms), put(es_s), put(ed_s)
    jax.block_until_ready((a1, a2, a3, a4))
    print("[prep] put", _t.time() - _t1, flush=True)
    _t2 = _t.time()
    nsc, dsc = run(a1, a2, a3, a4)
    jax.block_until_ready((nsc, dsc))
    print("[run] device", _t.time() - _t2, flush=True)
    node_scores = np.asarray(nsc).reshape(N)
    dag_scores = np.asarray(dsc).reshape(G, k)
    return node_scores, dag_scores


# revision 11
# speedup vs baseline: 1.2177x; 1.1379x over previous
import os
import numpy as np
import jax
import jax.numpy as jnp
from jax.sharding import Mesh, PartitionSpec as P
from jax.experimental.shard_map import shard_map
from functools import partial

# ---- constants (hardcoded from the problem spec) ----
N = 1_024_000
G = 4000
NPD = 256
LV = 8
E = 1_000_000
KEX = 50
D = 8
SLOPE = 0.01
NC = 8
NLOC = N // NC      # 128000 nodes per core
GLOC = G // NC      # 500 dags per core


def _mlp(params, x):
    n = len(params)
    for i, (W, b) in enumerate(params):
        x = x @ W + b
        if i < n - 1:
            x = jax.nn.leaky_relu(x, SLOPE)
    return x


def _bucket_edges(level_src, level_dst, level_mask):
    """Host-side: bucket each level's edges by the owner core of src (the
    aggregation target). Drop edges whose target has mask==0 at that level
    (their aggregate is multiplied by 0 in the residual update, so they are
    dead work). Pad all (level, core) lists to one common length."""
    src = np.asarray(level_src)
    dst = np.asarray(level_dst)
    msk = np.asarray(level_mask) > 0.5
    owner = src // NLOC
    keep = msk[np.arange(LV)[:, None], src]          # [LV, E] target-mask alive
    counts = np.zeros((LV, NC), np.int64)
    buckets = {}
    for l in range(LV):
        for c in range(NC):
            sel = np.nonzero((owner[l] == c) & keep[l])[0]
            buckets[(l, c)] = sel
            counts[l, c] = len(sel)
    emax = int(counts.max())
    emax = ((emax + 127) // 128) * 128
    s_out = np.zeros((LV, NC, emax), np.int32)       # local target id (padded -> NLOC sink)
    d_out = np.zeros((LV, NC, emax), np.int32)       # global source node
    b_out = np.zeros((LV, NC, NLOC + 1), np.int32)   # segment boundaries (edges sorted by target)
    for l in range(LV):
        for c in range(NC):
            sel = buckets[(l, c)]
            n = len(sel)
            st = src[l, sel] - c * NLOC
            o = np.argsort(st, kind="stable")
            sts = st[o]
            s_out[l, c, :n] = sts
            s_out[l, c, n:] = NLOC                   # sink row (dropped)
            d_out[l, c, :n] = dst[l, sel][o]
            b_out[l, c] = np.searchsorted(sts, np.arange(NLOC + 1))
    return s_out, d_out, b_out


def kernel(x, ptr, level_src, level_dst, level_mask,
           p_node_prep, p_node_msg, p_node_update, p_dag_msg,
           p_glob_msg, p_node_score, p_dag_score, num_executors):
    import time as _t
    _t0 = _t.time()
    x = np.asarray(x, np.float32)
    level_mask = np.asarray(level_mask, np.float32)
    k = int(num_executors)

    es, ed, eb = _bucket_edges(level_src, level_dst, level_mask)
    print("[prep] bucket", _t.time() - _t0, flush=True)

    params = [tuple((jnp.asarray(W), jnp.asarray(b)) for (W, b) in p)
              for p in (p_node_prep, p_node_msg, p_node_update, p_dag_msg,
                        p_glob_msg, p_node_score, p_dag_score)]
    (pp, pm, pu, pd, pg, pns, pds) = params

    devs = np.array(jax.devices()[:NC])
    mesh = Mesh(devs, ("i",))

    xs = x.reshape(NC, NLOC, 5)
    ms = level_mask.reshape(LV, NC, NLOC).transpose(1, 0, 2)     # [NC, LV, NLOC]
    es_s = es.transpose(1, 0, 2)                                  # [NC, LV, Emax]
    ed_s = ed.transpose(1, 0, 2)
    eb_s = eb.transpose(1, 0, 2)                                  # [NC, LV, NLOC+1]

    exec_feat = np.tile(np.arange(k, dtype=np.float32), GLOC)[:, None]  # [GLOC*K, 1]

    @partial(shard_map, mesh=mesh,
             in_specs=(P("i"), P("i"), P("i"), P("i"), P("i")),
             out_specs=(P("i"), P("i")))
    def run(xl, ml, esl, edl, ebl):
        xl = xl[0]; ml = ml[0]; esl = esl[0]; edl = edl[0]; ebl = ebl[0]
        h = _mlp(pp, xl)                                          # [NLOC, 8]
        for l in range(LV):
            y = _mlp(pm, h)                                       # [NLOC, 8]
            y_full = jax.lax.all_gather(y, "i", tiled=True)       # [N, 8]
            msgs = jnp.take(y_full, edl[l], axis=0)
            csum = jnp.cumsum(msgs, axis=0, dtype=jnp.float32)
            csum0 = jnp.concatenate([jnp.zeros((1, D), jnp.float32), csum], axis=0)
            agg = (jnp.take(csum0, ebl[l, 1:], axis=0)
                   - jnp.take(csum0, ebl[l, :-1], axis=0))
            upd = _mlp(pu, agg)
            h = h + ml[l][:, None] * upd
        dmsg = _mlp(pd, jnp.concatenate([xl, h], axis=1))         # [NLOC, 8]
        dsum = dmsg.reshape(GLOC, NPD, D).sum(axis=1)             # [GLOC, 8]
        gl = _mlp(pg, dsum).sum(axis=0, keepdims=True)            # [1, 8]
        glob = jax.lax.psum(gl, "i")                              # [1, 8]
        dag_b = jnp.broadcast_to(dsum[:, None, :], (GLOC, NPD, D)).reshape(NLOC, D)
        nin = jnp.concatenate([xl, h, dag_b,
                               jnp.broadcast_to(glob, (NLOC, D))], axis=1)
        nsc = _mlp(pns, nin)[:, 0]                                # [NLOC]
        dag_feat = xl.reshape(GLOC, NPD, 5)[:, 0, 0:3]            # [GLOC, 3]
        merged = jnp.concatenate([dag_feat, dsum], axis=1)        # [GLOC, 11]
        mrep = jnp.broadcast_to(merged[:, None, :], (GLOC, k, 11)).reshape(GLOC * k, 11)
        din = jnp.concatenate([mrep, jnp.broadcast_to(glob, (GLOC * k, D)),
                               jnp.asarray(exec_feat)], axis=1)
        dsc = _mlp(pds, din)[:, 0].reshape(GLOC, k)               # [GLOC, K]
        return nsc[None], dsc[None]

    sh = jax.sharding.NamedSharding(mesh, P("i"))

    def put(a):
        a = np.ascontiguousarray(a)
        shards = [jax.device_put(a[i:i + 1], devs[i]) for i in range(NC)]
        return jax.make_array_from_single_device_arrays(a.shape, sh, shards)

    _t1 = _t.time()
    a1, a2, a3, a4 = put(xs), put(ms), put(es_s), put(ed_s)
    jax.block_until_ready((a1, a2, a3, a4))
    print("[prep] put", _t.time() - _t1, flush=True)
    _t2 = _t.time()
    a5 = put(eb_s)
    nsc, dsc = run(a1, a2, a3, a4, a5)
    jax.block_until_ready((nsc, dsc))
    print("[run] first (trace+load+exec)", _t.time() - _t2, flush=True)
    _t3 = _t.time()
    nsc, dsc = run(a1, a2, a3, a4, a5)
    jax.block_until_ready((nsc, dsc))
    print("[run] second (pure exec)", _t.time() - _t3, flush=True)
    node_scores = np.asarray(nsc).reshape(N)
    dag_scores = np.asarray(dsc).reshape(G, k)
    return node_scores, dag_scores


# revision 12
# speedup vs baseline: 1.3323x; 1.0941x over previous
import os
import numpy as np
import jax
import jax.numpy as jnp
from jax.sharding import Mesh, PartitionSpec as P
from jax.experimental.shard_map import shard_map
from functools import partial

# ---- constants (hardcoded from the problem spec) ----
N = 1_024_000
G = 4000
NPD = 256
LV = 8
E = 1_000_000
KEX = 50
D = 8
SLOPE = 0.01
NC = 8
NLOC = N // NC      # 128000 nodes per core
GLOC = G // NC      # 500 dags per core


def _mlp(params, x):
    n = len(params)
    for i, (W, b) in enumerate(params):
        x = x @ W + b
        if i < n - 1:
            x = jax.nn.leaky_relu(x, SLOPE)
    return x


def _bucket_edges(level_src, level_dst, level_mask):
    """Host-side: bucket each level's edges by the owner core of src (the
    aggregation target). Drop edges whose target has mask==0 at that level
    (their aggregate is multiplied by 0 in the residual update, so they are
    dead work). Pad all (level, core) lists to one common length."""
    src = np.asarray(level_src)
    dst = np.asarray(level_dst)
    msk = np.asarray(level_mask) > 0.5
    owner = src // NLOC
    keep = msk[np.arange(LV)[:, None], src]          # [LV, E] target-mask alive
    counts = np.zeros((LV, NC), np.int64)
    buckets = {}
    for l in range(LV):
        for c in range(NC):
            sel = np.nonzero((owner[l] == c) & keep[l])[0]
            buckets[(l, c)] = sel
            counts[l, c] = len(sel)
    emax = int(counts.max())
    emax = ((emax + 127) // 128) * 128
    s_out = np.zeros((LV, NC, emax), np.int32)       # local target id (padded -> NLOC sink)
    d_out = np.zeros((LV, NC, emax), np.int32)       # global source node
    b_out = np.zeros((LV, NC, NLOC + 1), np.int32)   # segment boundaries (edges sorted by target)
    for l in range(LV):
        for c in range(NC):
            sel = buckets[(l, c)]
            n = len(sel)
            st = src[l, sel] - c * NLOC
            o = np.argsort(st, kind="stable")
            sts = st[o]
            s_out[l, c, :n] = sts
            s_out[l, c, n:] = NLOC                   # sink row (dropped)
            d_out[l, c, :n] = dst[l, sel][o]
            b_out[l, c] = np.searchsorted(sts, np.arange(NLOC + 1))
    return s_out, d_out, b_out


def kernel(x, ptr, level_src, level_dst, level_mask,
           p_node_prep, p_node_msg, p_node_update, p_dag_msg,
           p_glob_msg, p_node_score, p_dag_score, num_executors):
    import time as _t
    _t0 = _t.time()
    x = np.asarray(x, np.float32)
    level_mask = np.asarray(level_mask, np.float32)
    k = int(num_executors)

    es, ed, eb = _bucket_edges(level_src, level_dst, level_mask)
    print("[prep] bucket", _t.time() - _t0, flush=True)

    params = [tuple((jnp.asarray(W), jnp.asarray(b)) for (W, b) in p)
              for p in (p_node_prep, p_node_msg, p_node_update, p_dag_msg,
                        p_glob_msg, p_node_score, p_dag_score)]
    (pp, pm, pu, pd, pg, pns, pds) = params

    devs = np.array(jax.devices()[:NC])
    mesh = Mesh(devs, ("i",))

    xs = x.reshape(NC, NLOC, 5)
    ms = level_mask.reshape(LV, NC, NLOC).transpose(1, 0, 2)     # [NC, LV, NLOC]
    es_s = es.transpose(1, 0, 2)                                  # [NC, LV, Emax]
    ed_s = ed.transpose(1, 0, 2)
    eb_s = eb.transpose(1, 0, 2)                                  # [NC, LV, NLOC+1]

    exec_feat = np.tile(np.arange(k, dtype=np.float32), GLOC)[:, None]  # [GLOC*K, 1]
    emax0 = es.shape[2]
    nb0 = emax0 // 128
    tri128 = jnp.asarray(np.tril(np.ones((128, 128), np.float32)))
    triex = jnp.asarray(np.tril(np.ones((nb0, nb0), np.float32), k=-1))

    @partial(shard_map, mesh=mesh,
             in_specs=(P("i"), P("i"), P("i"), P("i"), P("i")),
             out_specs=(P("i"), P("i")))
    def run(xl, ml, esl, edl, ebl):
        xl = xl[0]; ml = ml[0]; esl = esl[0]; edl = edl[0]; ebl = ebl[0]
        h = _mlp(pp, xl)                                          # [NLOC, 8]
        for l in range(LV):
            y = _mlp(pm, h)                                       # [NLOC, 8]
            y_full = jax.lax.all_gather(y, "i", tiled=True)       # [N, 8]
            msgs = jnp.take(y_full, edl[l], axis=0)
            emax = msgs.shape[0]
            nb = emax // 128
            mb = msgs.reshape(nb, 128, D).transpose(1, 0, 2).reshape(128, nb * D)
            within = tri128 @ mb                                    # inclusive within-block
            wi = within.reshape(128, nb, D)
            bsum = wi[127]                                          # [nb, D] block sums
            boff = triex @ bsum                                     # exclusive block offsets
            csum = (wi + boff[None, :, :]).transpose(1, 0, 2).reshape(emax, D)
            csum0 = jnp.concatenate([jnp.zeros((1, D), jnp.float32), csum], axis=0)
            agg = (jnp.take(csum0, ebl[l, 1:], axis=0)
                   - jnp.take(csum0, ebl[l, :-1], axis=0))
            upd = _mlp(pu, agg)
            h = h + ml[l][:, None] * upd
        dmsg = _mlp(pd, jnp.concatenate([xl, h], axis=1))         # [NLOC, 8]
        dsum = dmsg.reshape(GLOC, NPD, D).sum(axis=1)             # [GLOC, 8]
        gl = _mlp(pg, dsum).sum(axis=0, keepdims=True)            # [1, 8]
        glob = jax.lax.psum(gl, "i")                              # [1, 8]
        dag_b = jnp.broadcast_to(dsum[:, None, :], (GLOC, NPD, D)).reshape(NLOC, D)
        nin = jnp.concatenate([xl, h, dag_b,
                               jnp.broadcast_to(glob, (NLOC, D))], axis=1)
        nsc = _mlp(pns, nin)[:, 0]                                # [NLOC]
        dag_feat = xl.reshape(GLOC, NPD, 5)[:, 0, 0:3]            # [GLOC, 3]
        merged = jnp.concatenate([dag_feat, dsum], axis=1)        # [GLOC, 11]
        mrep = jnp.broadcast_to(merged[:, None, :], (GLOC, k, 11)).reshape(GLOC * k, 11)
        din = jnp.concatenate([mrep, jnp.broadcast_to(glob, (GLOC * k, D)),
                               jnp.asarray(exec_feat)], axis=1)
        dsc = _mlp(pds, din)[:, 0].reshape(GLOC, k)               # [GLOC, K]
        return nsc[None], dsc[None]

    sh = jax.sharding.NamedSharding(mesh, P("i"))

    def put(a):
        a = np.ascontiguousarray(a)
        shards = [jax.device_put(a[i:i + 1], devs[i]) for i in range(NC)]
        return jax.make_array_from_single_device_arrays(a.shape, sh, shards)

    _t1 = _t.time()
    a1, a2, a3, a4 = put(xs), put(ms), put(es_s), put(ed_s)
    jax.block_until_ready((a1, a2, a3, a4))
    print("[prep] put", _t.time() - _t1, flush=True)
    _t2 = _t.time()
    a5 = put(eb_s)
    nsc, dsc = run(a1, a2, a3, a4, a5)
    jax.block_until_ready((nsc, dsc))
    print("[run] first (trace+load+exec)", _t.time() - _t2, flush=True)
    _t3 = _t.time()
    nsc, dsc = run(a1, a2, a3, a4, a5)
    jax.block_until_ready((nsc, dsc))
    print("[run] second (pure exec)", _t.time() - _t3, flush=True)
    node_scores = np.asarray(nsc).reshape(N)
    dag_scores = np.asarray(dsc).reshape(G, k)
    return node_scores, dag_scores
